# revision 10
# baseline (speedup 1.0000x reference)
"""ConvLSTM (Conv1D-LSTM over frames, sequential in time) on 8 NeuronCores.

Data-parallel over batch (8 per core). Per core the LSTM state is kept
transposed as (C=64, batch*frame) in SBUF so the recurrent 1-D conv becomes
PSUM-accumulated float32r matmuls with no per-step transpose:

  - 4 independent recurrent chains per core (one 2-batch group each, 512 gate
    columns) keep the PE densely fed (HAM-warm) and hide the per-step
    dependency chain.
  - h is stored padded (264 cols/batch); rows 64:128 hold a shift-by-1 copy
    so two conv taps contract per K=128 matmul (4 paired-tap matmuls).
  - the 9th tap, the z-conv (Cin=1), the bias, and the forget bias fold into
    one K=74 matmul over [h-slice; z-sliding-window; ones].

Engine rebalance vs the first version (Pool was at 94%, ACT 91%):
  - the K=74 h-slice copy is one SBUF-to-SBUF DMA (was DVE+Pool copies);
    it reads the previous step's h so the DMA latency is fully hidden.
  - groups 2/3 fold tanh(j) into the [j;o] sigmoid (j weights pre-scaled by
    2; tanh(j)=2*sigmoid(2j)-1 via one DVE tensor_scalar), saving one ACT op
    per group-step, and run the h'-multiply on Pool instead of DVE.
  - groups 0/1 compute the channel mean as a ones-vector PE matmul into a
    PSUM bank (emitted one step late so it never blocks gate matmuls in the
    in-order PE queue); groups 2/3 keep gpsimd partition_all_reduce.
"""
import sys
from contextlib import ExitStack

import numpy as np

if '/opt/trn_rl_repo' not in sys.path:
    sys.path.insert(0, '/opt/trn_rl_repo')

import concourse.bacc as bacc
import concourse.tile as tile
from concourse import bass_isa, mybir
from concourse.bass_utils import run_bass_kernel_spmd

B, T, F, C = 64, 64, 256, 64
NCORES, BL = 8, 8            # batches per core
NG = 4                       # groups (chains) per core, 2 batches each
COLS = F + 8                 # padded frame axis (4 each side)
W = 2 * F                    # free width per group (2 batches x 256)
F32 = mybir.dt.float32
F32R = mybir.dt.float32r
ACTF = mybir.ActivationFunctionType
ALU = mybir.AluOpType

_CACHE = {}


def _prep_weights(Wx, Wh, b):
    # gate reorder: [f, i, j, o]  (reference order: i, j, f, o)
    perm = np.concatenate([np.arange(128, 192), np.arange(0, 64),
                           np.arange(64, 128), np.arange(192, 256)])
    Whp = Wh[:, :, perm].astype(np.float32)            # (9, 64, 256)
    Wxp = Wx[:, 0, perm].astype(np.float32)            # (9, 256)
    bp = b[perm].astype(np.float32).copy()
    bp[0:64] += 1.0                                    # forget-gate bias
    whall = np.zeros((2, 128, 5, 256), np.float32)
    for p in range(4):
        whall[0, 0:64, p] = Whp[2 * p]
        whall[0, 64:128, p] = Whp[2 * p + 1]
    whall[0, 0:64, 4] = Whp[8]
    whall[0, 64:73, 4] = Wxp
    whall[0, 73, 4] = bp
    # variant 1 (groups 2/3): j columns scaled by 2 for the sigmoid(2j) trick
    whall[1] = whall[0]
    whall[1, :, :, 128:192] *= 2.0
    return np.ascontiguousarray(whall)


def _prep_core(z, h0, c0, core):
    zc = z[BL * core:BL * core + BL, :, :, 0]          # (8, T, F)
    h0c = h0[BL * core:BL * core + BL]                 # (8, F, C)
    c0c = c0[BL * core:BL * core + BL]

    zp = np.zeros((BL, T, COLS), np.float32)
    zp[:, :, 4:260] = zc
    zpa = np.ones((T, NG, 10, 2, 256), np.float32)
    for g in range(NG):
        for bb in range(2):
            bidx = 2 * g + bb
            for k in range(9):
                zpa[:, g, k, bb, :] = zp[bidx, :, k:k + 256]

    h0T = np.ascontiguousarray(h0c.transpose(2, 0, 1)).astype(np.float32)
    hh0 = np.zeros((2, NG, 128, 2, COLS), np.float32)
    for g in range(NG):
        for bb in range(2):
            hh0[0, g, 0:64, bb, 4:260] = h0T[:, 2 * g + bb, :]
    hh0[0, :, 64:128, :, 0:COLS - 1] = hh0[0, :, 0:64, :, 1:COLS]

    c0a = np.zeros((NG, 64, 2, 256), np.float32)
    for g in range(NG):
        for bb in range(2):
            c0a[g, :, bb, :] = c0c[2 * g + bb].T

    return {
        'zpa': np.ascontiguousarray(zpa.reshape(T, NG, 10, W)),
        'hh0': np.ascontiguousarray(hh0.reshape(2, NG, 128, 2 * COLS)),
        'c0a': np.ascontiguousarray(c0a.reshape(NG, 64, W)),
    }


def _build_program():
    nc = bacc.Bacc("TRN2", target_bir_lowering=False, debug=False,
                   enable_asserts=True, num_devices=NCORES)
    zpa_d = nc.dram_tensor("zpa", (T, NG, 10, W), F32R, kind="ExternalInput")
    hh0_d = nc.dram_tensor("hh0", (2, NG, 128, 2 * COLS), F32R,
                           kind="ExternalInput")
    c0a_d = nc.dram_tensor("c0a", (NG, 64, W), F32, kind="ExternalInput")
    wh_d = nc.dram_tensor("whall", (2, 128, 5, 256), F32R,
                          kind="ExternalInput")
    out_d = nc.dram_tensor("out", (64, 2, 2, 2, 256), F32, kind="ExternalOutput")

    with tile.TileContext(nc) as tc, ExitStack() as ctx:
        consts = ctx.enter_context(tc.tile_pool(name="consts", bufs=1))
        state = ctx.enter_context(tc.tile_pool(name="state", bufs=1))
        y_pool = ctx.enter_context(tc.tile_pool(name="ypool", bufs=4))
        ts_pool = ctx.enter_context(tc.tile_pool(name="tspool", bufs=4))
        m_pool = ctx.enter_context(tc.tile_pool(name="mpool", bufs=4))
        r8_pool = ctx.enter_context(tc.tile_pool(name="r8pool", bufs=12))
        srow_pool = ctx.enter_context(tc.tile_pool(name="srowpool", bufs=8))
        pg_pool = ctx.enter_context(tc.tile_pool(name="pgpool", bufs=7,
                                                 space="PSUM"))
        mb_pool = ctx.enter_context(tc.tile_pool(name="mbpool", bufs=1,
                                                 space="PSUM"))
        outs_pool = ctx.enter_context(tc.tile_pool(name="outs", bufs=1))

        wh_t = consts.tile([128, 2, 5, 256], F32R)
        nc.sync.dma_start(out=wh_t[:], in_=wh_d[:])
        ones64 = consts.tile([64, 1], F32R)
        nc.vector.memset(ones64[:], 1.0)

        hh = [[state.tile([128, 2, COLS], F32R, name=f"hh{par}{g}",
                          tag=f"hh{par}{g}")
               for g in range(NG)] for par in range(2)]
        # CJ[g]: rows 0:64 = c state (persistent), rows 64:128 = tanh(j)
        CJ = [state.tile([128, W], F32, name=f"CJ{g}", tag=f"CJ{g}")
              for g in range(NG)]
        for par in range(2):
            for g in range(NG):
                nc.sync.dma_start(out=hh[par][g][:], in_=hh0_d[par, g])
        for g in range(NG):
            nc.sync.dma_start(out=CJ[g][0:64, :], in_=c0a_d[g])

        outs_sb = outs_pool.tile([128, 1024], F32)
        MB = mb_pool.tile([128, 512], F32)   # PE-mean bank (groups 0/1)

        def pe_means(t):
            # channel means for groups 0/1 of step t, read from the parity
            # buffer h(t) lives in. Emitted two steps later so the matmuls
            # never stall the in-order PE queue behind fresh h updates.
            for g in range(2):
                nc.tensor.matmul(MB[32 * g:32 * g + 1, :], ones64[:],
                                 hh[(t + 1) % 2][g][0:64, :, 4:260],
                                 start=True, stop=True,
                                 tile_position=(0, 32 * g))
            sr2 = srow_pool.tile([2, W], F32, name="sr2", tag="sr2")
            nc.vector.tensor_copy(out=sr2[:], in_=MB[0:64:32, :])
            nc.sync.dma_start(out=outs_sb[t:t + 1, 0:1024], in_=sr2[:])

        # K=74 rhs tiles for step 0: z windows + h0-slice (dep: hh0 load)
        r8cur = []
        for g in range(NG):
            r8 = r8_pool.tile([80, 2, 256], F32R, name="r8p", tag="r8")
            nc.sync.dma_start(out=r8[64:74, :, :], in_=zpa_d[0, g])
            nc.sync.dma_start(out=r8[0:64, :, :], in_=hh[0][g][0:64, :, 8:264])
            r8cur.append(r8)
        srow_prev = [None] * NG

        for t in range(T):
            par, npar = t % 2, (t + 1) % 2
            if t >= 2:
                pe_means(t - 2)
            for g in range(2, NG):
                if srow_prev[g] is not None:
                    nc.sync.dma_start(
                        out=outs_sb[64 + t - 1:64 + t,
                                    (g % 2) * W:(g % 2) * W + W],
                        in_=srow_prev[g][0:1, :])
            r8nxt = [None] * NG
            for g in range(NG):
                wv = 0 if g < 2 else 1
                hcur, hnext = hh[par][g], hh[npar][g]
                r8 = r8cur[g]

                # P1 ([j; o]) first: its consumers get a head start. The two
                # K=74 matmuls (which consume the DMA-staged r8) go last so
                # the in-order PE queue never stalls on the r8 DMA.
                P1 = pg_pool.tile([128, W], F32, name="P1", tag="pg")
                P0 = pg_pool.tile([128, W], F32, name="P0", tag="pg")
                for m, P in ((1, P1), (0, P0)):
                    for tap in range(4):
                        nc.tensor.matmul(
                            P[:], wh_t[:, wv, tap, m * 128:(m + 1) * 128],
                            hcur[:, :, 2 * tap:2 * tap + 256],
                            start=(tap == 0), stop=False)
                for m, P in ((1, P1), (0, P0)):
                    nc.tensor.matmul(
                        P[:], wh_t[0:74, wv, 4, m * 128:(m + 1) * 128],
                        r8[0:74, :, :], start=False, stop=True)

                S = y_pool.tile([128, W], F32)
                if g < 2:
                    # tanh-j path: S = [sig f; sig i]; CJ[64:] = tanh j;
                    # TO = [tanh c | sig o] on rows 0:64
                    TO = ts_pool.tile([64, 2 * W], F32)
                    nc.scalar.activation(out=CJ[g][64:128, :],
                                         in_=P1[0:64, :], func=ACTF.Tanh)
                    nc.scalar.activation(out=TO[:, W:2 * W],
                                         in_=P1[64:128, :],
                                         func=ACTF.Sigmoid)
                    nc.scalar.activation(out=S[:], in_=P0[:],
                                         func=ACTF.Sigmoid)
                    MM = m_pool.tile([64, 2 * W], F32)
                    nc.gpsimd.tensor_mul(MM[:, W:2 * W], S[64:128, :],
                                         CJ[g][64:128, :])
                    nc.vector.tensor_mul(MM[:, 0:W], S[0:64, :],
                                         CJ[g][0:64, :])
                    nc.vector.tensor_add(CJ[g][0:64, :], MM[:, 0:W],
                                         MM[:, W:2 * W])
                    nc.scalar.activation(out=TO[:, 0:W], in_=CJ[g][0:64, :],
                                         func=ACTF.Tanh)
                    nc.vector.tensor_mul(hnext[0:64, :, 4:260],
                                         TO[:, 0:W], TO[:, W:2 * W])
                    nc.vector.tensor_copy(out=hnext[64:128, :, 3:259],
                                          in_=hnext[0:64, :, 4:260])
                    # channel mean via PE ones-matmul, emitted 2 steps later
                else:
                    # sigmoid(2j) path: S1 = sig([2j; o]) in one ACT op;
                    # tanh j = 2*sig(2j)-1 on DVE; tanh c written to rows
                    # 64:128 so the Pool h'-mul has same-base inputs.
                    S1 = y_pool.tile([128, W], F32, name="S1", tag="s1")
                    T2 = ts_pool.tile([128, W], F32, name="T2", tag="t2")
                    nc.scalar.activation(out=S1[:], in_=P1[:],
                                         func=ACTF.Sigmoid)
                    nc.gpsimd.tensor_scalar(CJ[g][64:128, :], S1[0:64, :],
                                            2.0, 1.0, ALU.mult, ALU.subtract)
                    nc.scalar.activation(out=S[:], in_=P0[:],
                                         func=ACTF.Sigmoid)
                    MM = m_pool.tile([64, 2 * W], F32)
                    nc.vector.tensor_mul(MM[:, W:2 * W], S[64:128, :],
                                         CJ[g][64:128, :])
                    nc.gpsimd.tensor_mul(MM[:, 0:W], S[0:64, :],
                                         CJ[g][0:64, :])
                    nc.vector.tensor_add(CJ[g][0:64, :], MM[:, 0:W],
                                         MM[:, W:2 * W])
                    nc.scalar.activation(out=T2[64:128, :],
                                         in_=CJ[g][0:64, :], func=ACTF.Tanh)
                    nc.vector.tensor_mul(hnext[0:64, :, 4:260],
                                         T2[64:128, :], S1[64:128, :])
                    nc.vector.tensor_copy(out=hnext[64:128, :, 3:259],
                                          in_=hnext[0:64, :, 4:260])
                    srow = srow_pool.tile([64, W], F32)
                    nc.gpsimd.partition_all_reduce(
                        srow[:], hnext[0:64, :, 4:260], channels=64,
                        reduce_op=bass_isa.ReduceOp.add)
                    srow_prev[g] = srow   # flushed at the next step's top

                # stage the next step's K=74 rhs right after h is final, so
                # the in-order SP/DMA queue never stalls on unmet deps
                if t + 1 < T:
                    r8n = r8_pool.tile([80, 2, 256], F32R, name="r8n",
                                       tag="r8")
                    nc.sync.dma_start(out=r8n[64:74, :, :],
                                      in_=zpa_d[t + 1, g])
                    nc.sync.dma_start(out=r8n[0:64, :, :],
                                      in_=hnext[0:64, :, 8:264])
                    r8nxt[g] = r8n
            r8cur = r8nxt

        pe_means(T - 2)
        pe_means(T - 1)
        for g in range(2, NG):
            nc.sync.dma_start(
                out=outs_sb[64 + T - 1:64 + T, (g % 2) * W:(g % 2) * W + W],
                in_=srow_prev[g][0:1, :])

        outs_tb = consts.tile([128, 1024], F32)
        nc.scalar.activation(out=outs_tb[:], in_=outs_sb[:], func=ACTF.Tanh,
                             scale=1.0 / 64.0)
        for gh in range(2):
            nc.sync.dma_start(out=out_d[:, gh],
                              in_=outs_tb[64 * gh:64 * gh + 64, :])

    nc.compile()
    return nc


def _get_program():
    if 'nc' not in _CACHE:
        _CACHE['nc'] = _build_program()
    return _CACHE['nc']


def kernel(z, h0, c0, Wx, Wh, b):
    z = np.asarray(z, np.float32)
    h0 = np.asarray(h0, np.float32)
    c0 = np.asarray(c0, np.float32)
    whall = _prep_weights(np.asarray(Wx, np.float32),
                          np.asarray(Wh, np.float32),
                          np.asarray(b, np.float32))
    in_maps = []
    for core in range(NCORES):
        m = _prep_core(z, h0, c0, core)
        m['whall'] = whall
        in_maps.append(m)
    nc = _get_program()
    res = run_bass_kernel_spmd(nc, in_maps, list(range(NCORES)))
    outs = []
    for core in range(NCORES):
        R = res.results[core]['out']        # (64, 2, 2, 2, 256) [t,gh,gl,bb,f]
        outs.append(R.transpose(1, 2, 3, 0, 4).reshape(BL, T * F))
    return np.concatenate(outs, axis=0)


# revision 22
# speedup vs baseline: 1.0858x; 1.0858x over previous
"""ConvLSTM (Conv1D-LSTM over frames, sequential in time) on 8 NeuronCores.

Data-parallel over batch (8 per core). Per core the LSTM state is kept
transposed as (C=64, batch*frame) in SBUF so the recurrent 1-D conv becomes
PSUM-accumulated float32r matmuls with no per-step transpose:

  - 4 independent recurrent chains per core (one 2-batch group each, 512 gate
    columns) keep the PE densely fed (HAM-warm) and hide the per-step
    dependency chain.
  - h is stored padded (264 cols/batch); rows 64:128 hold a shift-by-1 copy
    so two conv taps contract per K=128 matmul (4 paired-tap matmuls).
  - the 9th tap, the z-conv (Cin=1), the bias, and the forget bias fold into
    one K=74 matmul over [h-slice; z-sliding-window; ones].
  - gates are computed transposed as [f;i]/[j;o]; sigmoid(f,i) is one
    128-partition ACT op; the LSTM elementwise update runs as same-base
    64-partition ops split across DVE and GPSIMD.
  - the channel mean uses gpsimd partition_all_reduce (no PSUM extraction);
    one tanh(x/64) + output DMA runs at the end.
"""
import sys
from contextlib import ExitStack

import numpy as np

if '/opt/trn_rl_repo' not in sys.path:
    sys.path.insert(0, '/opt/trn_rl_repo')

import concourse.bacc as bacc
import concourse.tile as tile
from concourse import bass_isa, mybir
from concourse.bass_utils import run_bass_kernel_spmd

B, T, F, C = 64, 64, 256, 64
NCORES, BL = 8, 8            # batches per core
NG = 4                       # groups (chains) per core, 2 batches each
COLS = F + 8                 # padded frame axis (4 each side)
W = 2 * F                    # free width per group (2 batches x 256)
F32 = mybir.dt.float32
F32R = mybir.dt.float32r
ACTF = mybir.ActivationFunctionType

_CACHE = {}


def _prep_weights(Wx, Wh, b):
    # gate reorder: [f, i, j, o]  (reference order: i, j, f, o)
    perm = np.concatenate([np.arange(128, 192), np.arange(0, 64),
                           np.arange(64, 128), np.arange(192, 256)])
    Whp = Wh[:, :, perm].astype(np.float32)            # (9, 64, 256)
    Wxp = Wx[:, 0, perm].astype(np.float32)            # (9, 256)
    bp = b[perm].astype(np.float32).copy()
    bp[0:64] += 1.0                                    # forget-gate bias
    whall = np.zeros((128, 5, 256), np.float32)
    for p in range(4):
        whall[0:64, p] = Whp[2 * p]
        whall[64:128, p] = Whp[2 * p + 1]
    whall[0:64, 4] = Whp[8]
    whall[64:73, 4] = Wxp
    whall[73, 4] = bp
    return whall


def _prep_core(z, h0, c0, core):
    zc = z[BL * core:BL * core + BL, :, :, 0]          # (8, T, F)
    h0c = h0[BL * core:BL * core + BL]                 # (8, F, C)
    c0c = c0[BL * core:BL * core + BL]

    zp = np.zeros((BL, T, COLS), np.float32)
    zp[:, :, 4:260] = zc
    zpa = np.ones((T, NG, 10, 2, 256), np.float32)
    for g in range(NG):
        for bb in range(2):
            bidx = 2 * g + bb
            for k in range(9):
                zpa[:, g, k, bb, :] = zp[bidx, :, k:k + 256]

    h0T = np.ascontiguousarray(h0c.transpose(2, 0, 1)).astype(np.float32)
    hh0 = np.zeros((2, NG, 128, 2, COLS), np.float32)
    for g in range(NG):
        for bb in range(2):
            hh0[0, g, 0:64, bb, 4:260] = h0T[:, 2 * g + bb, :]
    hh0[0, :, 64:128, :, 0:COLS - 1] = hh0[0, :, 0:64, :, 1:COLS]

    c0a = np.zeros((NG, 64, 2, 256), np.float32)
    for g in range(NG):
        for bb in range(2):
            c0a[g, :, bb, :] = c0c[2 * g + bb].T

    # step-0 K=74 rhs fully host-packed: rows 0:64 = h0 slice, 64:74 = z+ones
    r80 = np.zeros((NG, 80, 2, 256), np.float32)
    r80[:, 0:64] = hh0[0, :, 0:64, :, 8:264]
    for g in range(NG):
        r80[g, 64:74] = zpa[0, g]
    return {
        'zpa': np.ascontiguousarray(zpa.reshape(T, NG, 10, W)),
        'hh0': np.ascontiguousarray(hh0.reshape(2, NG, 128, 2 * COLS)),
        'c0a': np.ascontiguousarray(c0a.reshape(NG, 64, W)),
        'r80': np.ascontiguousarray(r80),
    }


def _build_program():
    nc = bacc.Bacc("TRN2", target_bir_lowering=False, debug=False,
                   enable_asserts=True, num_devices=NCORES)
    zpa_d = nc.dram_tensor("zpa", (T, NG, 10, W), F32R, kind="ExternalInput")
    hh0_d = nc.dram_tensor("hh0", (2, NG, 128, 2 * COLS), F32R,
                           kind="ExternalInput")
    c0a_d = nc.dram_tensor("c0a", (NG, 64, W), F32, kind="ExternalInput")
    r80_d = nc.dram_tensor("r80", (NG, 80, 2, 256), F32R,
                           kind="ExternalInput")
    wh_d = nc.dram_tensor("whall", (128, 5, 256), F32R, kind="ExternalInput")
    out_d = nc.dram_tensor("out", (64, 2, 2, 2, 256), F32, kind="ExternalOutput")

    with tile.TileContext(nc) as tc, ExitStack() as ctx:
        consts = ctx.enter_context(tc.tile_pool(name="consts", bufs=1))
        state = ctx.enter_context(tc.tile_pool(name="state", bufs=1))
        y_pool = ctx.enter_context(tc.tile_pool(name="ypool", bufs=4))
        ts_pool = ctx.enter_context(tc.tile_pool(name="tspool", bufs=4))
        m_pool = ctx.enter_context(tc.tile_pool(name="mpool", bufs=4))
        r8_pool = ctx.enter_context(tc.tile_pool(name="r8pool", bufs=12))
        srow_pool = ctx.enter_context(tc.tile_pool(name="srowpool", bufs=8))
        pg_pool = ctx.enter_context(tc.tile_pool(name="pgpool", bufs=8,
                                                 space="PSUM"))
        outs_pool = ctx.enter_context(tc.tile_pool(name="outs", bufs=1))

        wh_t = consts.tile([128, 5, 256], F32R)
        nc.sync.dma_start(out=wh_t[:], in_=wh_d[:])

        hh = [[state.tile([128, 2, COLS], F32R, name=f"hh{par}{g}",
                          tag=f"hh{par}{g}")
               for g in range(NG)] for par in range(2)]
        # CJ[g]: rows 0:64 = c state (persistent), rows 64:128 = tanh(j)
        CJ = [state.tile([128, W], F32, name=f"CJ{g}", tag=f"CJ{g}")
              for g in range(NG)]
        for g in range(NG):
            nc.sync.dma_start(out=hh[0][g][:], in_=hh0_d[0, g])
        outs_sb = outs_pool.tile([128, 1024], F32)

        # PE warm-up: dummy matmuls on the weights ramp the tensor engine
        # to full clock before step 0's real matmuls dispatch
        Pwarm = pg_pool.tile([128, W], F32, name="Pwarm", tag="pg")
        for k in range(8):
            nc.tensor.matmul(Pwarm[:, 0:256], wh_t[:, k % 5, 0:128],
                             wh_t[:, (k + 1) % 5, :],
                             start=(k == 0), stop=(k == 9))

        r8cur = []
        for g in range(NG):
            r8 = r8_pool.tile([80, 2, 256], F32R, name="r8p", tag="r8")
            nc.sync.dma_start(out=r8[:], in_=r80_d[g])
            r8cur.append(r8)
        # c state is first needed ~mid-way through step 0's chain
        for g in range(NG):
            nc.sync.dma_start(out=CJ[g][0:64, :], in_=c0a_d[g])
        # parity-1 buffers only need their zero padding before step 0's
        # h'-write; load them after everything step-0-critical
        for g in range(NG):
            nc.sync.dma_start(out=hh[1][g][:], in_=hh0_d[1, g])

        for t in range(T):
            par, npar = t % 2, (t + 1) % 2
            r8nxt = [None] * NG
            for g in range(NG):
                hcur, hnext = hh[par][g], hh[npar][g]
                r8 = r8cur[g]

                # P1 ([j; o]) first: tanh_j -> multB gets a head start.
                P1 = pg_pool.tile([128, W], F32, name="P1", tag="pg")
                P0 = pg_pool.tile([128, W], F32, name="P0", tag="pg")
                for m, P in ((1, P1), (0, P0)):
                    for tap in range(4):
                        nc.tensor.matmul(
                            P[:], wh_t[:, tap, m * 128:(m + 1) * 128],
                            hcur[:, :, 2 * tap:2 * tap + 256],
                            start=(tap == 0), stop=False)
                    nc.tensor.matmul(
                        P[:], wh_t[0:74, 4, m * 128:(m + 1) * 128],
                        r8[0:74, :, :], start=False, stop=True)

                # S = [sig f; sig i] from P0; CJ[64:] = tanh j from P1
                S = y_pool.tile([128, W], F32)
                TO = ts_pool.tile([64, 2 * W], F32)
                nc.scalar.activation(out=CJ[g][64:128, :], in_=P1[0:64, :],
                                     func=ACTF.Tanh)
                nc.scalar.activation(out=S[:], in_=P0[:], func=ACTF.Sigmoid)
                nc.scalar.activation(out=TO[:, W:2 * W], in_=P1[64:128, :],
                                     func=ACTF.Sigmoid)
                # MM = [c*sig f | sig i * tanh j] side by side on rows 0:64
                MM = m_pool.tile([64, 2 * W], F32)
                nc.vector.tensor_mul(MM[:, W:2 * W], S[64:128, :],
                                     CJ[g][64:128, :])
                nc.gpsimd.tensor_mul(MM[:, 0:W], S[0:64, :], CJ[g][0:64, :])
                nc.vector.tensor_add(CJ[g][0:64, :], MM[:, 0:W], MM[:, W:2 * W])
                # TO = [tanh c | sig o] on rows 0:64
                nc.scalar.activation(out=TO[:, 0:W], in_=CJ[g][0:64, :],
                                     func=ACTF.Tanh)
                nc.vector.tensor_mul(hnext[0:64, :, 4:260],
                                     TO[:, 0:W], TO[:, W:2 * W])
                nc.vector.tensor_copy(out=hnext[64:128, :, 3:259],
                                      in_=hnext[0:64, :, 4:260])
                # channel mean via gpsimd partition all-reduce
                srow = srow_pool.tile([64, W], F32)
                nc.gpsimd.partition_all_reduce(
                    srow[:], hnext[0:64, :, 4:260], channels=64,
                    reduce_op=bass_isa.ReduceOp.add)
                if t + 1 < T:
                    r8n = r8_pool.tile([80, 2, 256], F32R, name="r8n",
                                       tag="r8")
                    nc.sync.dma_start(out=r8n[64:74, :, :],
                                      in_=zpa_d[t + 1, g])
                    nc.sync.dma_start(out=r8n[0:64, :, :],
                                      in_=hnext[0:64, :, 8:264])
                    r8nxt[g] = r8n
                nc.sync.dma_start(
                    out=outs_sb[64 * (g // 2) + t:64 * (g // 2) + t + 1,
                                (g % 2) * W:(g % 2) * W + W],
                    in_=srow[0:1, :])
            r8cur = r8nxt

        outs_tb = consts.tile([128, 1024], F32)
        nc.scalar.activation(out=outs_tb[:], in_=outs_sb[:], func=ACTF.Tanh,
                             scale=1.0 / 64.0)
        for gh in range(2):
            nc.sync.dma_start(out=out_d[:, gh],
                              in_=outs_tb[64 * gh:64 * gh + 64, :])

    nc.compile()
    return nc


def _get_program():
    if 'nc' not in _CACHE:
        _CACHE['nc'] = _build_program()
    return _CACHE['nc']


def kernel(z, h0, c0, Wx, Wh, b):
    z = np.asarray(z, np.float32)
    h0 = np.asarray(h0, np.float32)
    c0 = np.asarray(c0, np.float32)
    whall = _prep_weights(np.asarray(Wx, np.float32),
                          np.asarray(Wh, np.float32),
                          np.asarray(b, np.float32))
    in_maps = []
    for core in range(NCORES):
        m = _prep_core(z, h0, c0, core)
        m['whall'] = whall
        in_maps.append(m)
    nc = _get_program()
    res = run_bass_kernel_spmd(nc, in_maps, list(range(NCORES)))
    outs = []
    for core in range(NCORES):
        R = res.results[core]['out']        # (64, 2, 2, 2, 256) [t,gh,gl,bb,f]
        outs.append(R.transpose(1, 2, 3, 0, 4).reshape(BL, T * F))
    return np.concatenate(outs, axis=0)

